# revision 38
# baseline (speedup 1.0000x reference)
"""Causal multi-head attention (B=2, T=2048, C=1024, H=16) on 8 TRN2 cores.

Sharding: data-parallel over batch (2 groups of 4 cores), tensor-parallel
over heads within a group (4 heads / core). Each core:
  1. computes Q^T, K^T (layout [d, t]) and V (layout [t, d]) for its heads
     from x[b]^T (host-transposed) and its W column slices,
  2. runs causal attention in the S^T = K @ Q^T orientation (softmax sums
     come for free from a ones-column appended to V; softmax max-subtraction
     is skipped -- scores are O(1) here so exp is safe),
  3. AllGathers the per-head attention outputs across all 8 cores in
     four per-q-chunk quarter collectives,
  4. computes a 256-column slice of the output projection over its own
     batch's 1024 gathered features.
Host reassembles the 8 [2048, 256] shards into [2, 2048, 1024].

Perf notes vs the first working version (322.8us -> ~255-270us):
  - the AllGather is split into four quarter-collectives (one per q-chunk)
    over all 8 cores (Shared output = the fast collective path), so only
    the last quarter's gather latency sits on the critical-path tail.
  - the output projection contracts over this batch's 1024 gathered rows
    only, instead of 2048 rows half-zeroed (per-core dynamic DRAM offset
    from partition_id; one 3-dim mega-DMA per block amortizes the ~2.3us
    dynamic-DMA register/scratch preamble).
  - softmax normalization is pair-batched: one [2, 512] iterative
    reciprocal per head pair (cost scales with FREE elements only), one
    indicator-block matmul broadcasts both heads' reciprocals, and the
    PE-touching half is deferred into the next pair's S-stream so the
    in-order PE queue never parks behind the reciprocal latency.
  - causal-mask multiplies only touch the columns that can be masked
    ((dm+1)*128 of 512 per diagonal k-tile) and alternate between the
    GpSimd and Vector engines so they serialize on neither.
  - diagonal-tile exp skips the fully-masked column prefix (the mask
    multiply zeroes it; the pT pool is zero-initialized once so stale
    bits are finite).
  - V-projection bias is folded into the output bias host-side
    (bo' = bo + bv @ Wo).
  - weight DMAs ride the Activation engine's HWDGE queue in parallel with
    x^T on the Sync queue; wq/wk/wv are packed into one [C, 768] tensor
    (one DMA per c-tile instead of three).
  - Q projection mt=0 is emitted c-tile-outer so the PE starts ~2us into
    the x^T load; the remaining projections and V t-blocks are interleaved
    with early attention pairs to fill pipeline-fill bubbles.

Matmul operands are bf16 (fp32 PSUM accumulation).
"""

import os
import sys

import numpy as np
import ml_dtypes

for _p in ("/opt/trn_rl_repo",):
    if os.path.isdir(_p) and _p not in sys.path:
        sys.path.insert(0, _p)

import concourse.bacc as bacc
import concourse.mybir as mybir
import concourse.tile as tile
from concourse import bass_utils
from concourse.ap import AP

B, T, C, H, D = 2, 2048, 1024, 16, 64
NCORES = 8
GP = 4              # cores per batch group
HPC = H // GP       # heads per core = 4
DS = HPC * D        # per-core head-dim slice = 256
NCT = C // 128      # c-tiles = 8
NQC = T // 512      # q-chunks = 4
NKT = T // 128      # k-tiles = 16

F32 = mybir.dt.float32
BF16 = mybir.dt.bfloat16
AF = mybir.ActivationFunctionType
ALU = mybir.AluOpType
NPBF = ml_dtypes.bfloat16

_PROG = None
LAST_RESULTS = None  # BassKernelResults of the most recent run (for test.py)


def _normalize_recip(nc, pn, ops, hp):
    """Vector-only first half of the pair normalize: stage both heads'
    row-sums and compute one [2, 512] reciprocal (the iterative
    reciprocal costs ~6 cycles per FREE element regardless of
    partitions, so one call covers the pair). Emitted right at pair end
    -- nothing here touches the PE queue."""
    sums = pn.tile([65, 512], F32, tag="sums")
    nc.vector.tensor_copy(sums[64:65, :], ops[hp][64:65, :])
    nc.vector.tensor_copy(sums[0:1, :], ops[hp + 1][64:65, :])
    recip_bf = pn.tile([65, 512], BF16, tag="recipbf")
    with nc.allow_low_precision(reason="normalize denominators"):
        nc.vector.reciprocal(recip_bf[0:65, :], sums[0:65, :])
    return recip_bf


def _normalize_apply(nc, pn, psB, ones_t, attn_sb, ops, recip_bf, hp, qc):
    """Second half: one matmul against the indicator block (row 64 of
    ones_t selects output partitions 0..63 for head hp, row 0 selects
    64..127 for head hp+1) broadcasts the reciprocals, then two
    multiplies write the normalized attention. Deferred into the next
    pair's S-stream so the PE queue never waits on the reciprocal."""
    bc_ps = psB.tile([128, 512], F32, tag="bcs")
    nc.tensor.matmul(
        bc_ps[:, :],
        ones_t[0:65, :],
        recip_bf[0:65, :],
        start=True,
        stop=True,
    )
    bc_sb = pn.tile([128, 512], F32, tag="bc")
    nc.vector.tensor_copy(bc_sb[:, :], bc_ps[:, :])
    for j, h in enumerate((hp, hp + 1)):
        aslc = attn_sb[(h, qc // 2)][:, 512 * (qc % 2) : 512 * (qc % 2 + 1)]
        nc.vector.tensor_tensor(
            aslc, ops[h][0:64, :], bc_sb[64 * j : 64 * (j + 1), :], ALU.mult
        )


def _emit(nc, tc, io):
    (xT, wqkv, wo, bq2, bk2, bo_bc, maskd, onesd, onesv, out_shard) = io

    ag_in = [
        nc.dram_tensor(f"ag_in{i}", [DS, 512], BF16) for i in range(NQC)
    ]
    ag_out = [
        nc.dram_tensor(
            f"ag_out{i}", [NCORES * DS, 512], BF16, addr_space="Shared"
        )
        for i in range(NQC)
    ]

    with (
        tc.tile_pool(name="outer", bufs=1) as po,
    ):
        # ---- persistent tiles; weights ride the scalar HWDGE queue ----
        bq_sb = po.tile([128, 2], F32, tag="bq")
        nc.scalar.dma_start(bq_sb[:, :], bq2[:, :])
        bk_sb = po.tile([128, 2], F32, tag="bk")
        nc.scalar.dma_start(bk_sb[:, :], bk2[:, :])
        ones_t = po.tile([128, 128], BF16, tag="ones")
        nc.scalar.dma_start(ones_t[:, :], onesd[:, :])
        mask_sb = po.tile([128, 4 * 512], BF16, tag="mask")
        attn_sb = {}
        for h in range(HPC):
            for hf in (0, 1):
                attn_sb[(h, hf)] = po.tile(
                    [64, T // 2], BF16, tag=f"attn{h}_{hf}", name=f"attn{h}_{hf}"
                )

        with (
            tc.tile_pool(name="proj", bufs=1) as pp,
            tc.tile_pool(name="work", bufs=16) as pw,
            tc.tile_pool(name="nrm", bufs=2) as pn,
            tc.tile_pool(name="fin", bufs=1) as pf,
            tc.tile_pool(name="ao", bufs=2) as pao,
            tc.tile_pool(name="osb", bufs=3) as posb,
            tc.tile_pool(name="psA", bufs=5, space="PSUM") as psA,
            tc.tile_pool(name="psO", bufs=1, space="PSUM") as psO,
            tc.tile_pool(name="psB", bufs=1, space="PSUM") as psB,
        ):
            # zero-init the pT pool slots: the diagonal exp skips fully
            # masked column prefixes, and the mask multiply's 0 * stale-bits
            # must not see NaN/Inf bit patterns on the first rotation.
            for i in range(16):
                pT_init = pw.tile([128, 512], BF16, tag="pT", name=f"pT_init{i}")
                nc.vector.memset(pT_init[:, :], 0.0)

            # the pair-normalize reciprocal sweeps partitions 0..64 of the
            # sums tiles; rows 1..63 are never written, so memset both pool
            # slots to 1.0 once (1/1=1, zeroed by the indicator matmul; left
            # as bit-garbage they could be NaN/0 and the matmul's 0*Inf
            # would poison the broadcast).
            for i in range(2):
                s_init = pn.tile([65, 512], F32, tag="sums", name=f"sums_init{i}")
                nc.vector.memset(s_init[0:65, :], 1.0)

            # ---- load x^T (sync queue) and packed wqkv (scalar queue),
            # interleaved per c-tile so the Q projection can start while
            # later tiles are still in flight
            xT_sb, w_sb = [], []
            for ci in range(NCT):
                t_ = pp.tile([128, T], BF16, tag=f"xt{ci}", name=f"xt{ci}")
                nc.sync.dma_start(t_[:, :], xT[128 * ci : 128 * (ci + 1), :])
                xT_sb.append(t_)
                t_ = pp.tile([128, 3 * DS], BF16, tag=f"w{ci}", name=f"w{ci}")
                nc.scalar.dma_start(t_[:, :], wqkv[128 * ci : 128 * (ci + 1), :])
                w_sb.append(t_)
            # remaining constants on the scalar queue (needed later)
            vp_sb = pp.tile([128, HPC * NKT * 65], BF16, tag="vp")
            nc.scalar.dma_start(vp_sb[:, :], onesv[:, :])
            nc.scalar.dma_start(mask_sb[:, :], maskd[:, :])
            wo_sb = []
            for ci in range(NCT):
                t_ = pf.tile([128, DS], BF16, tag=f"wo{ci}", name=f"wo{ci}")
                nc.scalar.dma_start(t_[:, :], wo[128 * ci : 128 * (ci + 1), :])
                wo_sb.append(t_)
            bo_sb = pf.tile([128, DS], F32, tag="bo")
            nc.scalar.dma_start(bo_sb[:, :], bo_bc[:, :])

            # ---- Q^T projection, c-tile-outer: the accumulation for all
            # 4 t-chunks of an mt block proceeds as x^T tiles arrive, so
            # the PE starts ~2us into the x^T load. Two [128,1024] PSUM
            # pair-slots hold the 4 chunk accumulators.
            qT_sb = [
                pp.tile([128, T], BF16, tag=f"qT{mt}", name=f"qT{mt}")
                for mt in range(2)
            ]
            kT_sb = [
                pp.tile([128, T], BF16, tag=f"kT{mt}", name=f"kT{mt}")
                for mt in range(2)
            ]
            def q_proj_ci_outer(mt):
                # c-tile-outer: accumulation proceeds as x^T tiles arrive,
                # so the PE starts ~2us into the x^T load (mt=0 only; by
                # mt=1 the DMAs are done and t-chunk-outer is cheaper on
                # PSUM slots).
                slots = [
                    psA.tile([128, 512], F32, tag="sps", name=f"qacc{mt}_{i}")
                    for i in range(NQC)
                ]
                for ci in range(NCT):
                    for tch in range(NQC):
                        nc.tensor.matmul(
                            slots[tch][:, :],
                            w_sb[ci][:, 128 * mt : 128 * (mt + 1)],
                            xT_sb[ci][:, 512 * tch : 512 * (tch + 1)],
                            start=(ci == 0),
                            stop=(ci == NCT - 1),
                        )
                for tch in range(NQC):
                    nc.vector.tensor_scalar_add(
                        qT_sb[mt][:, 512 * tch : 512 * (tch + 1)],
                        slots[tch][:, :],
                        bq_sb[:, mt : mt + 1],
                    )

            def qk_proj_tch(dst, woff, bias, mt):
                for tch in range(NQC):
                    ps = psA.tile([128, 512], F32, tag="sps")
                    for ci in range(NCT):
                        nc.tensor.matmul(
                            ps[:, :],
                            w_sb[ci][:, woff + 128 * mt : woff + 128 * (mt + 1)],
                            xT_sb[ci][:, 512 * tch : 512 * (tch + 1)],
                            start=(ci == 0),
                            stop=(ci == NCT - 1),
                        )
                    nc.vector.tensor_scalar_add(
                        dst[mt][:, 512 * tch : 512 * (tch + 1)],
                        ps[:, :],
                        bias[:, mt : mt + 1],
                    )

            # ---- V projection: out [t, d] + ones column ----------------
            def v_proj_block(tts):
                for tt in tts:
                    ps = psA.tile([128, 512], F32, tag="sps", name=f"vps{tt}")
                    for ci in range(NCT):
                        nc.tensor.matmul(
                            ps[:, 0:DS],
                            xT_sb[ci][:, 128 * tt : 128 * (tt + 1)],
                            w_sb[ci][:, 2 * DS : 3 * DS],
                            start=(ci == 0),
                            stop=(ci == NCT - 1),
                        )
                    for h in range(HPC):
                        nc.vector.tensor_copy(
                            vp_sb[:, 1040 * h + 65 * tt : 1040 * h + 65 * tt + 64],
                            ps[:, 64 * h : 64 * (h + 1)],
                        )

            # ---- causal attention (S^T orientation) --------------------
            # Head pairs share each k-tile step; PV accumulations are
            # emitted as same-bank-adjacent pairs one k-step behind the
            # S/exp front. V-projection t-blocks are interleaved between
            # q-chunks (each chunk only consumes V up to its causal k
            # range), so the attention pipeline fill overlaps V matmuls.
            # normalize + quarter-AllGather for a finished pair; the caller
            # defers this into the NEXT pair's S-stream so the PE queue is
            # never parked behind the reciprocal chain.
            def flush_pair(pend):
                qc, hp, ops, recip_bf = pend
                _normalize_apply(
                    nc, pn, psB, ones_t, attn_sb, ops, recip_bf, hp, qc
                )
                if hp == 2:
                    # both head pairs of this q-chunk are normalized: ship
                    # the quarter AllGather (Shared output = fast path).
                    for h in range(HPC):
                        nc.sync.dma_start(
                            ag_in[qc][64 * h : 64 * (h + 1), :],
                            attn_sb[(h, qc // 2)][
                                :, 512 * (qc % 2) : 512 * (qc % 2 + 1)
                            ],
                        )
                    nc.gpsimd.collective_compute(
                        "AllGather",
                        ALU.bypass,
                        replica_groups=[list(range(NCORES))],
                        ins=[ag_in[qc][:, :]],
                        outs=[ag_out[qc][:, :]],
                    )

            pending = None

            def attn_pair(qc, hp):
                nonlocal pending
                nkt = 4 * qc + 4
                if True:
                    ops = {}
                    for h in (hp, hp + 1):
                        ops[h] = psO.tile(
                            [65, 512], F32, tag=f"ops{h % 2}", name=f"op_q{qc}h{h}"
                        )
                    pTs = {}
                    for kt in range(nkt + 2):
                        if kt == 3 and pending is not None:
                            flush_pair(pending)
                            pending = None
                        if kt < nkt:
                            for h in (hp, hp + 1):
                                mt, pof = h // 2, 64 * (h % 2)
                                qs = qT_sb[mt][pof : pof + 64, 512 * qc : 512 * (qc + 1)]
                                st = psA.tile([128, 512], F32, tag="sps")
                                nc.tensor.matmul(
                                    st[:, :],
                                    kT_sb[mt][pof : pof + 64, 128 * kt : 128 * (kt + 1)],
                                    qs,
                                    start=True,
                                    stop=True,
                                )
                                pT = pw.tile([128, 512], BF16, tag="pT")
                                dm = kt - 4 * qc
                                # diagonal tiles: columns below dm*128 are
                                # fully masked -> skip them in the exp (the
                                # mask multiply zeroes them; the pT pool is
                                # zero-initialized so stale bits are finite)
                                e0 = dm * 128 if dm > 0 else 0
                                nc.scalar.activation(
                                    pT[:, e0:512], st[:, e0:512], AF.Exp, scale=0.125
                                )
                                if dm >= 0:  # mask cols that can hide
                                    mw = (dm + 1) * 128
                                    # alternate engines: the 4 diagonal
                                    # masks arrive back-to-back at chunk
                                    # end and would serialize on one queue
                                    eng = nc.gpsimd if dm % 2 == 0 else nc.vector
                                    eng.tensor_tensor(
                                        pT[:, 0:mw],
                                        pT[:, 0:mw],
                                        mask_sb[:, 512 * dm : 512 * dm + mw],
                                        ALU.mult,
                                    )
                                pTs[(h, kt)] = pT
                        # PV pairs, emitted same-bank adjacent, one pair
                        # of k-steps behind the S/exp front
                        if kt % 2 == 1 and kt - 1 - 2 >= 0:
                            kv = kt - 1 - 2
                            # alternate heads so consecutive accumulations
                            # hit different PSUM banks (same-bank
                            # back-to-back accumulation stalls the PE
                            # pipeline on the read-modify-write)
                            for k2 in (kv, kv + 1):
                                for h in (hp, hp + 1):
                                    nc.tensor.matmul(
                                        ops[h][:, :],
                                        vp_sb[:, 1040 * h + 65 * k2 : 1040 * h + 65 * k2 + 65],
                                        pTs.pop((h, k2))[:, :],
                                        start=(k2 == 0),
                                        stop=(k2 == nkt - 1),
                                    )
                    # drain remaining PV steps (last two k-tiles)
                    for k2 in sorted(set(k for (hh, k) in pTs)):
                        for h in (hp, hp + 1):
                            nc.tensor.matmul(
                                ops[h][:, :],
                                vp_sb[:, 1040 * h + 65 * k2 : 1040 * h + 65 * k2 + 65],
                                pTs.pop((h, k2))[:, :],
                                start=(k2 == 0),
                                stop=(k2 == nkt - 1),
                            )
                    recip_bf = _normalize_recip(nc, pn, ops, hp)
                    pending = (qc, hp, ops, recip_bf)

            # ---- schedule: interleave the remaining projections and V
            # blocks with early attention pairs (pair (0,0) only needs the
            # mt=0 projections and V t-tiles 0..3), so proj matmuls fill
            # the attention pipeline-fill bubbles.
            q_proj_ci_outer(0)
            qk_proj_tch(kT_sb, DS, bk_sb, 0)
            v_proj_block(range(0, 4))
            attn_pair(0, 0)
            qk_proj_tch(qT_sb, 0, bq_sb, 1)
            qk_proj_tch(kT_sb, DS, bk_sb, 1)
            attn_pair(0, 2)
            v_proj_block(range(4, 8))
            attn_pair(1, 0)
            attn_pair(1, 2)
            v_proj_block(range(8, 16))
            attn_pair(2, 0)
            attn_pair(2, 2)
            attn_pair(3, 0)
            attn_pair(3, 2)
            flush_pair(pending)

            # ---- output projection: full T, 256-column slice of Wo over
            # this batch's 1024 gathered features. The batch's rows sit at
            # a per-core offset (0 or 1024 rows); one dynamic-offset 3-dim
            # mega-DMA per quarter pulls all 8 c-tiles (the dynamic-DMA
            # register/scratch preamble is ~2.3us, so amortize it).
            for tg in range(NQC):
                ao = pao.tile([128, NCT * 512], BF16, tag="ao")
                dq = nc.scalar if tg == 3 else nc.sync
                boff = (dq.partition_id() // GP) * (C * 512)
                base = ag_out[tg][0:128, :]
                in_ap = AP(
                    base.tensor,
                    base.offset + boff,
                    [[512, 128], [128 * 512, NCT], [1, 512]],
                )
                sb = ao[:, :]
                out_ap = AP(
                    sb.tensor, sb.offset, [[NCT * 512, 128], [512, NCT], [1, 512]]
                )
                dq.dma_start(out_ap, in_ap)
                osb = posb.tile([128, 4 * DS], F32, tag="osb")
                for tj in range(4):
                    tt = 4 * tg + tj
                    ps = psO.tile(
                        [128, DS], F32, tag=f"ops{tt % 2}", name=f"out_ps{tt}"
                    )
                    for ci in range(NCT):
                        nc.tensor.matmul(
                            ps[:, :],
                            ao[:, 512 * ci + 128 * tj : 512 * ci + 128 * (tj + 1)],
                            wo_sb[ci][:, :],
                            start=(ci == 0),
                            stop=(ci == NCT - 1),
                        )
                    nc.vector.tensor_tensor(
                        osb[:, DS * tj : DS * (tj + 1)], ps[:, :], bo_sb[:, :],
                        ALU.add,
                    )
                st_base = out_shard[512 * tg : 512 * tg + 128, :]
                st_out = AP(
                    st_base.tensor,
                    st_base.offset,
                    [[DS, 128], [128 * DS, 4], [1, DS]],
                )
                st_sb = osb[:, :]
                st_in = AP(
                    st_sb.tensor, st_sb.offset, [[4 * DS, 128], [DS, 4], [1, DS]]
                )
                nc.sync.dma_start(st_out, st_in)


def _build_program():
    nc = bacc.Bacc(
        "TRN2",
        target_bir_lowering=False,
        debug=False,
        num_devices=NCORES,
    )
    xT = nc.dram_tensor("xT", [C, T], BF16, kind="ExternalInput")
    wqkv = nc.dram_tensor("wqkv", [C, 3 * DS], BF16, kind="ExternalInput")
    wo = nc.dram_tensor("wo", [C, DS], BF16, kind="ExternalInput")
    bq2 = nc.dram_tensor("bq2", [128, 2], F32, kind="ExternalInput")
    bk2 = nc.dram_tensor("bk2", [128, 2], F32, kind="ExternalInput")
    bo_bc = nc.dram_tensor("bo_bc", [128, DS], F32, kind="ExternalInput")
    maskd = nc.dram_tensor("maskd", [128, 4 * 512], BF16, kind="ExternalInput")
    onesd = nc.dram_tensor("onesd", [128, 128], BF16, kind="ExternalInput")
    onesv = nc.dram_tensor(
        "onesv", [128, HPC * NKT * 65], BF16, kind="ExternalInput"
    )
    out_shard = nc.dram_tensor("out_shard", [T, DS], F32, kind="ExternalOutput")
    io = (xT, wqkv, wo, bq2, bk2, bo_bc, maskd, onesd, onesv, out_shard)
    with tile.TileContext(nc) as tc:
        _emit(nc, tc, io)
    nc.compile()
    return nc


def _make_indicator():
    # row 64 selects the low 64 output partitions (head hp) and row 0 the
    # high 64 (head hp+1) for the paired softmax-denominator broadcast
    # matmul; other rows are zero so the stray reciprocal outputs on
    # partitions 1..63 (finite, via the memset-to-1 of the sums tiles)
    # cannot contribute.
    ind = np.zeros((128, 128), np.float32)
    ind[64, 0:64] = 1.0
    ind[0, 64:128] = 1.0
    return ind.astype(NPBF)


def _make_mask():
    # multiplicative causal mask blocks for the 4 diagonal positions:
    # 1 where k is visible (128*m + k_local <= q_local), 0 otherwise
    k = np.arange(128, dtype=np.int64)[:, None]
    q = np.arange(512, dtype=np.int64)[None, :]
    mask = np.zeros((128, 4 * 512), np.float32)
    for m in range(4):
        mask[:, 512 * m : 512 * (m + 1)] = (128 * m + k <= q).astype(np.float32)
    return mask.astype(NPBF)


def _make_in_maps(x, Wq, bq, Wk, bk, Wv, bv, Wo, bo):
    mask = _make_mask()
    in_maps = []
    for c in range(NCORES):
        b, g = c // GP, c % GP
        hs = slice(DS * g, DS * (g + 1))
        wqkv = np.concatenate([Wq[:, hs], Wk[:, hs], Wv[:, hs]], axis=1)
        bo_eff = bo[hs] + bv @ Wo[:, hs]  # V bias folded through Wo
        in_maps.append(
            {
                "xT": np.ascontiguousarray(x[b].T).astype(NPBF),
                "wqkv": np.ascontiguousarray(wqkv).astype(NPBF),
                "wo": np.ascontiguousarray(Wo[:, hs]).astype(NPBF),
                "bq2": np.ascontiguousarray(bq[hs].reshape(2, 128).T),
                "bk2": np.ascontiguousarray(bk[hs].reshape(2, 128).T),
                "bo_bc": np.tile(bo_eff[None, :], (128, 1)).astype(np.float32),
                "maskd": mask,
                "onesd": _make_indicator(),
                "onesv": np.ones((128, HPC * NKT * 65), NPBF),
            }
        )
    return in_maps


def kernel(x, Wq, bq, Wk, bk, Wv, bv, Wo, bo, _trace=False, _trace_cores=None):
    global _PROG, LAST_RESULTS
    x = np.asarray(x, np.float32)
    Wq, bq = np.asarray(Wq, np.float32), np.asarray(bq, np.float32)
    Wk, bk = np.asarray(Wk, np.float32), np.asarray(bk, np.float32)
    Wv, bv = np.asarray(Wv, np.float32), np.asarray(bv, np.float32)
    Wo, bo = np.asarray(Wo, np.float32), np.asarray(bo, np.float32)

    if _PROG is None:
        _PROG = _build_program()
    nc = _PROG

    in_maps = _make_in_maps(x, Wq, bq, Wk, bk, Wv, bv, Wo, bo)

    kw = {}
    if _trace:
        kw["trace"] = True
        if _trace_cores is not None:
            kw["trace_cores"] = _trace_cores
    res = bass_utils.run_bass_kernel_spmd(nc, in_maps, list(range(NCORES)), **kw)
    LAST_RESULTS = res

    out = np.empty((B, T, C), np.float32)
    for c in range(NCORES):
        b, g = c // GP, c % GP
        out[b, :, DS * g : DS * (g + 1)] = res.results[c]["out_shard"]
    return out


# revision 41
# speedup vs baseline: 1.2011x; 1.2011x over previous
"""Causal multi-head attention (B=2, T=2048, C=1024, H=16) on 8 TRN2 cores.

Sharding: data-parallel over batch (2 groups of 4 cores), tensor-parallel
over heads within a group (4 heads / core). Each core:
  1. computes Q^T, K^T (layout [d, t]) and V (layout [t, d]) for its heads
     from x[b]^T (host-transposed) and its W column slices,
  2. runs causal attention in the S^T = K @ Q^T orientation (softmax sums
     come for free from a ones-column appended to V; softmax max-subtraction
     is skipped -- scores are O(1) here so exp is safe),
  3. AllGathers the per-head attention outputs across all 8 cores in
     four per-q-chunk quarter collectives,
  4. computes a 256-column slice of the output projection over its own
     batch's 1024 gathered features.
Host reassembles the 8 [2048, 256] shards into [2, 2048, 1024].

Perf notes vs the first working version (322.8us -> ~255-270us):
  - the AllGather is split into four quarter-collectives (one per q-chunk)
    over all 8 cores (Shared output = the fast collective path), so only
    the last quarter's gather latency sits on the critical-path tail.
  - the output projection contracts over this batch's 1024 gathered rows
    only, instead of 2048 rows half-zeroed (per-core dynamic DRAM offset
    from partition_id; one 3-dim mega-DMA per block amortizes the ~2.3us
    dynamic-DMA register/scratch preamble).
  - softmax normalization is pair-batched: one [2, 512] iterative
    reciprocal per head pair (cost scales with FREE elements only), one
    indicator-block matmul broadcasts both heads' reciprocals, and the
    PE-touching half is deferred into the next pair's S-stream so the
    in-order PE queue never parks behind the reciprocal latency.
  - causal-mask multiplies only touch the columns that can be masked
    ((dm+1)*128 of 512 per diagonal k-tile) and alternate between the
    GpSimd and Vector engines so they serialize on neither.
  - diagonal-tile exp skips the fully-masked column prefix (the mask
    multiply zeroes it; the pT pool is zero-initialized once so stale
    bits are finite).
  - V-projection bias is folded into the output bias host-side
    (bo' = bo + bv @ Wo).
  - weight DMAs ride the Activation engine's HWDGE queue in parallel with
    x^T on the Sync queue; wq/wk/wv are packed into one [C, 768] tensor
    (one DMA per c-tile instead of three).
  - Q projection mt=0 is emitted c-tile-outer so the PE starts ~2us into
    the x^T load; the remaining projections and V t-blocks are interleaved
    with early attention pairs to fill pipeline-fill bubbles.

Matmul operands are bf16 (fp32 PSUM accumulation).
"""

import os
import sys

import numpy as np
import ml_dtypes

for _p in ("/opt/trn_rl_repo",):
    if os.path.isdir(_p) and _p not in sys.path:
        sys.path.insert(0, _p)

import concourse.bacc as bacc
import concourse.mybir as mybir
import concourse.tile as tile
from concourse import bass_utils
from concourse.ap import AP

B, T, C, H, D = 2, 2048, 1024, 16, 64
NCORES = 8
GP = 4              # cores per batch group
HPC = H // GP       # heads per core = 4
DS = HPC * D        # per-core head-dim slice = 256
NCT = C // 128      # c-tiles = 8
NQC = T // 512      # q-chunks = 4
NKT = T // 128      # k-tiles = 16

F32 = mybir.dt.float32
BF16 = mybir.dt.bfloat16
AF = mybir.ActivationFunctionType
ALU = mybir.AluOpType
NPBF = ml_dtypes.bfloat16

_PROG = None
LAST_RESULTS = None  # BassKernelResults of the most recent run (for test.py)


def _normalize_recip(nc, pn, ops, hp):
    """Vector-only first half of the pair normalize: stage both heads'
    row-sums and compute one [2, 512] reciprocal (the iterative
    reciprocal costs ~6 cycles per FREE element regardless of
    partitions, so one call covers the pair). Emitted right at pair end
    -- nothing here touches the PE queue."""
    sums = pn.tile([65, 512], F32, tag="sums")
    nc.vector.tensor_copy(sums[64:65, :], ops[hp][64:65, :])
    nc.vector.tensor_copy(sums[0:1, :], ops[hp + 1][64:65, :])
    recip_bf = pn.tile([65, 512], BF16, tag="recipbf")
    with nc.allow_low_precision(reason="normalize denominators"):
        nc.vector.reciprocal(recip_bf[0:65, :], sums[0:65, :])
    return recip_bf


def _normalize_apply(nc, pn, psB, ones_t, attn_sb, ops, recip_bf, hp, qc):
    """Second half: one matmul against the indicator block (row 64 of
    ones_t selects output partitions 0..63 for head hp, row 0 selects
    64..127 for head hp+1) broadcasts the reciprocals, then two
    multiplies write the normalized attention. Deferred into the next
    pair's S-stream so the PE queue never waits on the reciprocal."""
    bc_ps = psB.tile([128, 512], F32, tag="bcs")
    nc.tensor.matmul(
        bc_ps[:, :],
        ones_t[0:65, :],
        recip_bf[0:65, :],
        start=True,
        stop=True,
    )
    bc_sb = pn.tile([128, 512], F32, tag="bc")
    nc.vector.tensor_copy(bc_sb[:, :], bc_ps[:, :])
    for j, h in enumerate((hp, hp + 1)):
        aslc = attn_sb[(h, qc // 2)][:, 512 * (qc % 2) : 512 * (qc % 2 + 1)]
        nc.vector.tensor_tensor(
            aslc, ops[h][0:64, :], bc_sb[64 * j : 64 * (j + 1), :], ALU.mult
        )


def _emit(nc, tc, io):
    (xT, wqkv, wo, bq2, bk2, bo_bc, maskd, onesd, onesv, out_shard) = io

    ag_in = [
        nc.dram_tensor(f"ag_in{i}", [DS, 512], BF16) for i in range(NQC)
    ]
    ag_out = [
        nc.dram_tensor(
            f"ag_out{i}", [NCORES * DS, 512], BF16, addr_space="Shared"
        )
        for i in range(NQC)
    ]

    with (
        tc.tile_pool(name="outer", bufs=1) as po,
    ):
        # ---- persistent tiles; weights ride the scalar HWDGE queue ----
        bq_sb = po.tile([128, 2], F32, tag="bq")
        nc.scalar.dma_start(bq_sb[:, :], bq2[:, :])
        bk_sb = po.tile([128, 2], F32, tag="bk")
        nc.scalar.dma_start(bk_sb[:, :], bk2[:, :])
        ones_t = po.tile([128, 128], BF16, tag="ones")
        nc.scalar.dma_start(ones_t[:, :], onesd[:, :])
        mask_sb = po.tile([128, 4 * 512], BF16, tag="mask")
        attn_sb = {}
        for h in range(HPC):
            for hf in (0, 1):
                attn_sb[(h, hf)] = po.tile(
                    [64, T // 2], BF16, tag=f"attn{h}_{hf}", name=f"attn{h}_{hf}"
                )

        with (
            tc.tile_pool(name="proj", bufs=1) as pp,
            tc.tile_pool(name="work", bufs=16) as pw,
            tc.tile_pool(name="nrm", bufs=2) as pn,
            tc.tile_pool(name="fin", bufs=1) as pf,
            tc.tile_pool(name="ao", bufs=2) as pao,
            tc.tile_pool(name="osb", bufs=3) as posb,
            tc.tile_pool(name="psA", bufs=5, space="PSUM") as psA,
            tc.tile_pool(name="psO", bufs=1, space="PSUM") as psO,
            tc.tile_pool(name="psB", bufs=1, space="PSUM") as psB,
        ):
            # zero-init the pT pool slots: the diagonal exp skips fully
            # masked column prefixes, and the mask multiply's 0 * stale-bits
            # must not see NaN/Inf bit patterns on the first rotation.
            for i in range(16):
                pT_init = pw.tile([128, 512], BF16, tag="pT", name=f"pT_init{i}")
                nc.vector.memset(pT_init[:, :], 0.0)

            # the pair-normalize reciprocal sweeps partitions 0..64 of the
            # sums tiles; rows 1..63 are never written, so memset both pool
            # slots to 1.0 once (1/1=1, zeroed by the indicator matmul; left
            # as bit-garbage they could be NaN/0 and the matmul's 0*Inf
            # would poison the broadcast).
            for i in range(2):
                s_init = pn.tile([65, 512], F32, tag="sums", name=f"sums_init{i}")
                nc.vector.memset(s_init[0:65, :], 1.0)

            # ---- load x^T (sync queue) and packed wqkv (scalar queue),
            # interleaved per c-tile so the Q projection can start while
            # later tiles are still in flight
            xT_sb, w_sb = [], []
            for ci in range(NCT):
                t_ = pp.tile([128, T], BF16, tag=f"xt{ci}", name=f"xt{ci}")
                nc.sync.dma_start(t_[:, :], xT[128 * ci : 128 * (ci + 1), :])
                xT_sb.append(t_)
                t_ = pp.tile([128, 3 * DS], BF16, tag=f"w{ci}", name=f"w{ci}")
                nc.scalar.dma_start(t_[:, :], wqkv[128 * ci : 128 * (ci + 1), :])
                w_sb.append(t_)
            # remaining constants on the scalar queue (needed later)
            vp_sb = pp.tile([128, HPC * NKT * 65], BF16, tag="vp")
            nc.scalar.dma_start(vp_sb[:, :], onesv[:, :])
            nc.scalar.dma_start(mask_sb[:, :], maskd[:, :])
            wo_sb = []
            for ci in range(NCT):
                t_ = pf.tile([128, DS], BF16, tag=f"wo{ci}", name=f"wo{ci}")
                nc.scalar.dma_start(t_[:, :], wo[128 * ci : 128 * (ci + 1), :])
                wo_sb.append(t_)
            bo_sb = pf.tile([128, DS], F32, tag="bo")
            nc.scalar.dma_start(bo_sb[:, :], bo_bc[:, :])

            # ---- Q^T projection, c-tile-outer: the accumulation for all
            # 4 t-chunks of an mt block proceeds as x^T tiles arrive, so
            # the PE starts ~2us into the x^T load. Two [128,1024] PSUM
            # pair-slots hold the 4 chunk accumulators.
            qT_sb = [
                pp.tile([128, T], BF16, tag=f"qT{mt}", name=f"qT{mt}")
                for mt in range(2)
            ]
            kT_sb = [
                pp.tile([128, T], BF16, tag=f"kT{mt}", name=f"kT{mt}")
                for mt in range(2)
            ]
            def q_proj_ci_outer(mt):
                # c-tile-outer: accumulation proceeds as x^T tiles arrive,
                # so the PE starts ~2us into the x^T load (mt=0 only; by
                # mt=1 the DMAs are done and t-chunk-outer is cheaper on
                # PSUM slots).
                slots = [
                    psA.tile([128, 512], F32, tag="sps", name=f"qacc{mt}_{i}")
                    for i in range(NQC)
                ]
                for ci in range(NCT):
                    for tch in range(NQC):
                        nc.tensor.matmul(
                            slots[tch][:, :],
                            w_sb[ci][:, 128 * mt : 128 * (mt + 1)],
                            xT_sb[ci][:, 512 * tch : 512 * (tch + 1)],
                            start=(ci == 0),
                            stop=(ci == NCT - 1),
                        )
                for tch in range(NQC):
                    nc.vector.tensor_scalar_add(
                        qT_sb[mt][:, 512 * tch : 512 * (tch + 1)],
                        slots[tch][:, :],
                        bq_sb[:, mt : mt + 1],
                    )

            def qk_proj_tch(dst, woff, bias, mt):
                for tch in range(NQC):
                    ps = psA.tile([128, 512], F32, tag="sps")
                    for ci in range(NCT):
                        nc.tensor.matmul(
                            ps[:, :],
                            w_sb[ci][:, woff + 128 * mt : woff + 128 * (mt + 1)],
                            xT_sb[ci][:, 512 * tch : 512 * (tch + 1)],
                            start=(ci == 0),
                            stop=(ci == NCT - 1),
                        )
                    nc.vector.tensor_scalar_add(
                        dst[mt][:, 512 * tch : 512 * (tch + 1)],
                        ps[:, :],
                        bias[:, mt : mt + 1],
                    )

            # ---- V projection: out [t, d] + ones column ----------------
            def v_proj_block(tts):
                for tt in tts:
                    ps = psA.tile([128, 512], F32, tag="sps", name=f"vps{tt}")
                    for ci in range(NCT):
                        nc.tensor.matmul(
                            ps[:, 0:DS],
                            xT_sb[ci][:, 128 * tt : 128 * (tt + 1)],
                            w_sb[ci][:, 2 * DS : 3 * DS],
                            start=(ci == 0),
                            stop=(ci == NCT - 1),
                        )
                    for h in range(HPC):
                        nc.vector.tensor_copy(
                            vp_sb[:, 1040 * h + 65 * tt : 1040 * h + 65 * tt + 64],
                            ps[:, 64 * h : 64 * (h + 1)],
                        )

            # ---- causal attention (S^T orientation) --------------------
            # Head pairs share each k-tile step; PV accumulations are
            # emitted as same-bank-adjacent pairs one k-step behind the
            # S/exp front. V-projection t-blocks are interleaved between
            # q-chunks (each chunk only consumes V up to its causal k
            # range), so the attention pipeline fill overlaps V matmuls.
            # normalize + quarter-AllGather for a finished pair; the caller
            # defers this into the NEXT pair's S-stream so the PE queue is
            # never parked behind the reciprocal chain.
            def flush_pair(pend):
                qc, hp, ops, recip_bf = pend
                _normalize_apply(
                    nc, pn, psB, ones_t, attn_sb, ops, recip_bf, hp, qc
                )
                if hp == 2:
                    # both head pairs of this q-chunk are normalized: ship
                    # the quarter AllGather (Shared output = fast path).
                    for h in range(HPC):
                        nc.sync.dma_start(
                            ag_in[qc][64 * h : 64 * (h + 1), :],
                            attn_sb[(h, qc // 2)][
                                :, 512 * (qc % 2) : 512 * (qc % 2 + 1)
                            ],
                        )
                    nc.gpsimd.collective_compute(
                        "AllGather",
                        ALU.bypass,
                        replica_groups=[list(range(NCORES))],
                        ins=[ag_in[qc][:, :]],
                        outs=[ag_out[qc][:, :]],
                    )

            pending = None

            def attn_pair(qc, hp):
                nonlocal pending
                nkt = 4 * qc + 4
                if True:
                    ops = {}
                    for h in (hp, hp + 1):
                        ops[h] = psO.tile(
                            [65, 512], F32, tag=f"ops{h % 2}", name=f"op_q{qc}h{h}"
                        )
                    pTs = {}
                    for kt in range(nkt + 2):
                        if kt == 3 and pending is not None:
                            flush_pair(pending)
                            pending = None
                        if kt < nkt:
                            for h in (hp, hp + 1):
                                mt, pof = h // 2, 64 * (h % 2)
                                qs = qT_sb[mt][pof : pof + 64, 512 * qc : 512 * (qc + 1)]
                                st = psA.tile([128, 512], F32, tag="sps")
                                nc.tensor.matmul(
                                    st[:, :],
                                    kT_sb[mt][pof : pof + 64, 128 * kt : 128 * (kt + 1)],
                                    qs,
                                    start=True,
                                    stop=True,
                                )
                                pT = pw.tile([128, 512], BF16, tag="pT")
                                dm = kt - 4 * qc
                                # diagonal tiles: columns below dm*128 are
                                # fully masked -> skip them in the exp (the
                                # mask multiply zeroes them; the pT pool is
                                # zero-initialized so stale bits are finite)
                                e0 = dm * 128 if dm > 0 else 0
                                nc.scalar.activation(
                                    pT[:, e0:512], st[:, e0:512], AF.Exp, scale=0.125
                                )
                                if dm >= 0:  # mask cols that can hide
                                    mw = (dm + 1) * 128
                                    # alternate engines: the 4 diagonal
                                    # masks arrive back-to-back at chunk
                                    # end and would serialize on one queue
                                    eng = nc.gpsimd if dm % 2 == 0 else nc.vector
                                    eng.tensor_tensor(
                                        pT[:, 0:mw],
                                        pT[:, 0:mw],
                                        mask_sb[:, 512 * dm : 512 * dm + mw],
                                        ALU.mult,
                                    )
                                pTs[(h, kt)] = pT
                        # PV pairs, emitted same-bank adjacent, one pair
                        # of k-steps behind the S/exp front
                        if kt % 2 == 1 and kt - 1 - 2 >= 0:
                            kv = kt - 1 - 2
                            # alternate heads so consecutive accumulations
                            # hit different PSUM banks (same-bank
                            # back-to-back accumulation stalls the PE
                            # pipeline on the read-modify-write)
                            for k2 in (kv, kv + 1):
                                for h in (hp, hp + 1):
                                    nc.tensor.matmul(
                                        ops[h][:, :],
                                        vp_sb[:, 1040 * h + 65 * k2 : 1040 * h + 65 * k2 + 65],
                                        pTs.pop((h, k2))[:, :],
                                        start=(k2 == 0),
                                        stop=(k2 == nkt - 1),
                                    )
                    # drain remaining PV steps (last two k-tiles)
                    for k2 in sorted(set(k for (hh, k) in pTs)):
                        for h in (hp, hp + 1):
                            nc.tensor.matmul(
                                ops[h][:, :],
                                vp_sb[:, 1040 * h + 65 * k2 : 1040 * h + 65 * k2 + 65],
                                pTs.pop((h, k2))[:, :],
                                start=(k2 == 0),
                                stop=(k2 == nkt - 1),
                            )
                    recip_bf = _normalize_recip(nc, pn, ops, hp)
                    pending = (qc, hp, ops, recip_bf)

            # ---- schedule: interleave the remaining projections and V
            # blocks with early attention pairs (pair (0,0) only needs the
            # mt=0 projections and V t-tiles 0..3), so proj matmuls fill
            # the attention pipeline-fill bubbles.
            q_proj_ci_outer(0)
            qk_proj_tch(kT_sb, DS, bk_sb, 0)
            v_proj_block(range(0, 4))
            attn_pair(0, 0)
            qk_proj_tch(qT_sb, 0, bq_sb, 1)
            qk_proj_tch(kT_sb, DS, bk_sb, 1)
            attn_pair(0, 2)
            v_proj_block(range(4, 8))
            attn_pair(1, 0)
            attn_pair(1, 2)
            v_proj_block(range(8, 16))
            attn_pair(2, 0)
            attn_pair(2, 2)
            attn_pair(3, 0)
            attn_pair(3, 2)
            flush_pair(pending)

            # ---- output projection: full T, 256-column slice of Wo over
            # this batch's 1024 gathered features. The batch's rows sit at
            # a per-core offset (0 or 1024 rows); one dynamic-offset 3-dim
            # mega-DMA per quarter pulls all 8 c-tiles (the dynamic-DMA
            # register/scratch preamble is ~2.3us, so amortize it).
            for tg in range(NQC):
                ao = pao.tile([128, NCT * 512], BF16, tag="ao")
                dq = nc.scalar if tg == 3 else nc.sync
                # the mega-DMA below reads ag_out through a dynamic-offset
                # AP, which the dependency tracker may not tie to the
                # collective's write; this static-offset read IS tracked,
                # and the engine queue is in-order, so it fences the
                # dynamic read behind collective completion.
                fence = pao.tile([1, 64], BF16, tag="agfence")
                dq.dma_start(fence[:, :], ag_out[tg][1024:1025, 0:64])
                boff = (dq.partition_id() // GP) * (C * 512)
                base = ag_out[tg][0:128, :]
                in_ap = AP(
                    base.tensor,
                    base.offset + boff,
                    [[512, 128], [128 * 512, NCT], [1, 512]],
                )
                sb = ao[:, :]
                out_ap = AP(
                    sb.tensor, sb.offset, [[NCT * 512, 128], [512, NCT], [1, 512]]
                )
                dq.dma_start(out_ap, in_ap)
                osb = posb.tile([128, 4 * DS], F32, tag="osb")
                for tj in range(4):
                    tt = 4 * tg + tj
                    ps = psO.tile(
                        [128, DS], F32, tag=f"ops{tt % 2}", name=f"out_ps{tt}"
                    )
                    for ci in range(NCT):
                        nc.tensor.matmul(
                            ps[:, :],
                            ao[:, 512 * ci + 128 * tj : 512 * ci + 128 * (tj + 1)],
                            wo_sb[ci][:, :],
                            start=(ci == 0),
                            stop=(ci == NCT - 1),
                        )
                    nc.vector.tensor_tensor(
                        osb[:, DS * tj : DS * (tj + 1)], ps[:, :], bo_sb[:, :],
                        ALU.add,
                    )
                st_base = out_shard[512 * tg : 512 * tg + 128, :]
                st_out = AP(
                    st_base.tensor,
                    st_base.offset,
                    [[DS, 128], [128 * DS, 4], [1, DS]],
                )
                st_sb = osb[:, :]
                st_in = AP(
                    st_sb.tensor, st_sb.offset, [[4 * DS, 128], [DS, 4], [1, DS]]
                )
                nc.sync.dma_start(st_out, st_in)


def _build_program():
    nc = bacc.Bacc(
        "TRN2",
        target_bir_lowering=False,
        debug=False,
        num_devices=NCORES,
    )
    xT = nc.dram_tensor("xT", [C, T], BF16, kind="ExternalInput")
    wqkv = nc.dram_tensor("wqkv", [C, 3 * DS], BF16, kind="ExternalInput")
    wo = nc.dram_tensor("wo", [C, DS], BF16, kind="ExternalInput")
    bq2 = nc.dram_tensor("bq2", [128, 2], F32, kind="ExternalInput")
    bk2 = nc.dram_tensor("bk2", [128, 2], F32, kind="ExternalInput")
    bo_bc = nc.dram_tensor("bo_bc", [128, DS], F32, kind="ExternalInput")
    maskd = nc.dram_tensor("maskd", [128, 4 * 512], BF16, kind="ExternalInput")
    onesd = nc.dram_tensor("onesd", [128, 128], BF16, kind="ExternalInput")
    onesv = nc.dram_tensor(
        "onesv", [128, HPC * NKT * 65], BF16, kind="ExternalInput"
    )
    out_shard = nc.dram_tensor("out_shard", [T, DS], F32, kind="ExternalOutput")
    io = (xT, wqkv, wo, bq2, bk2, bo_bc, maskd, onesd, onesv, out_shard)
    with tile.TileContext(nc) as tc:
        _emit(nc, tc, io)
    nc.compile()
    return nc


def _make_indicator():
    # row 64 selects the low 64 output partitions (head hp) and row 0 the
    # high 64 (head hp+1) for the paired softmax-denominator broadcast
    # matmul; other rows are zero so the stray reciprocal outputs on
    # partitions 1..63 (finite, via the memset-to-1 of the sums tiles)
    # cannot contribute.
    ind = np.zeros((128, 128), np.float32)
    ind[64, 0:64] = 1.0
    ind[0, 64:128] = 1.0
    return ind.astype(NPBF)


def _make_mask():
    # multiplicative causal mask blocks for the 4 diagonal positions:
    # 1 where k is visible (128*m + k_local <= q_local), 0 otherwise
    k = np.arange(128, dtype=np.int64)[:, None]
    q = np.arange(512, dtype=np.int64)[None, :]
    mask = np.zeros((128, 4 * 512), np.float32)
    for m in range(4):
        mask[:, 512 * m : 512 * (m + 1)] = (128 * m + k <= q).astype(np.float32)
    return mask.astype(NPBF)


def _make_in_maps(x, Wq, bq, Wk, bk, Wv, bv, Wo, bo):
    mask = _make_mask()
    in_maps = []
    for c in range(NCORES):
        b, g = c // GP, c % GP
        hs = slice(DS * g, DS * (g + 1))
        wqkv = np.concatenate([Wq[:, hs], Wk[:, hs], Wv[:, hs]], axis=1)
        bo_eff = bo[hs] + bv @ Wo[:, hs]  # V bias folded through Wo
        in_maps.append(
            {
                "xT": np.ascontiguousarray(x[b].T).astype(NPBF),
                "wqkv": np.ascontiguousarray(wqkv).astype(NPBF),
                "wo": np.ascontiguousarray(Wo[:, hs]).astype(NPBF),
                "bq2": np.ascontiguousarray(bq[hs].reshape(2, 128).T),
                "bk2": np.ascontiguousarray(bk[hs].reshape(2, 128).T),
                "bo_bc": np.tile(bo_eff[None, :], (128, 1)).astype(np.float32),
                "maskd": mask,
                "onesd": _make_indicator(),
                "onesv": np.ones((128, HPC * NKT * 65), NPBF),
            }
        )
    return in_maps


def kernel(x, Wq, bq, Wk, bk, Wv, bv, Wo, bo, _trace=False, _trace_cores=None):
    global _PROG, LAST_RESULTS
    x = np.asarray(x, np.float32)
    Wq, bq = np.asarray(Wq, np.float32), np.asarray(bq, np.float32)
    Wk, bk = np.asarray(Wk, np.float32), np.asarray(bk, np.float32)
    Wv, bv = np.asarray(Wv, np.float32), np.asarray(bv, np.float32)
    Wo, bo = np.asarray(Wo, np.float32), np.asarray(bo, np.float32)

    if _PROG is None:
        _PROG = _build_program()
    nc = _PROG

    in_maps = _make_in_maps(x, Wq, bq, Wk, bk, Wv, bv, Wo, bo)

    kw = {}
    if _trace:
        kw["trace"] = True
        if _trace_cores is not None:
            kw["trace_cores"] = _trace_cores
    res = bass_utils.run_bass_kernel_spmd(nc, in_maps, list(range(NCORES)), **kw)
    LAST_RESULTS = res

    out = np.empty((B, T, C), np.float32)
    for c in range(NCORES):
        b, g = c // GP, c % GP
        out[b, :, DS * g : DS * (g + 1)] = res.results[c]["out_shard"]
    return out


# revision 44
# speedup vs baseline: 1.2435x; 1.0353x over previous
"""Causal multi-head attention (B=2, T=2048, C=1024, H=16) on 8 TRN2 cores.

Sharding: data-parallel over batch (2 groups of 4 cores), tensor-parallel
over heads within a group (4 heads / core). Each core:
  1. computes Q^T, K^T (layout [d, t]) and V (layout [t, d]) for its heads
     from x[b]^T (host-transposed) and its W column slices,
  2. runs causal attention in the S^T = K @ Q^T orientation (softmax sums
     come for free from a ones-column appended to V; softmax max-subtraction
     is skipped -- scores are O(1) here so exp is safe),
  3. AllGathers the per-head attention outputs across all 8 cores in
     four per-q-chunk quarter collectives,
  4. computes a 256-column slice of the output projection over its own
     batch's 1024 gathered features.
Host reassembles the 8 [2048, 256] shards into [2, 2048, 1024].

Perf notes vs the first working version (322.8us -> ~255-270us):
  - the AllGather is split into four quarter-collectives (one per q-chunk)
    over all 8 cores (Shared output = the fast collective path), so only
    the last quarter's gather latency sits on the critical-path tail.
  - the output projection contracts over this batch's 1024 gathered rows
    only, instead of 2048 rows half-zeroed (per-core dynamic DRAM offset
    from partition_id; one 3-dim mega-DMA per block amortizes the ~2.3us
    dynamic-DMA register/scratch preamble).
  - softmax normalization is pair-batched: one [2, 512] iterative
    reciprocal per head pair (cost scales with FREE elements only), one
    indicator-block matmul broadcasts both heads' reciprocals, and the
    PE-touching half is deferred into the next pair's S-stream so the
    in-order PE queue never parks behind the reciprocal latency.
  - causal-mask multiplies only touch the columns that can be masked
    ((dm+1)*128 of 512 per diagonal k-tile) and alternate between the
    GpSimd and Vector engines so they serialize on neither.
  - diagonal-tile exp skips the fully-masked column prefix (the mask
    multiply zeroes it; the pT pool is zero-initialized once so stale
    bits are finite).
  - V-projection bias is folded into the output bias host-side
    (bo' = bo + bv @ Wo).
  - weight DMAs ride the Activation engine's HWDGE queue in parallel with
    x^T on the Sync queue; wq/wk/wv are packed into one [C, 768] tensor
    (one DMA per c-tile instead of three).
  - Q projection mt=0 is emitted c-tile-outer so the PE starts ~2us into
    the x^T load; the remaining projections and V t-blocks are interleaved
    with early attention pairs to fill pipeline-fill bubbles.

Matmul operands are bf16 (fp32 PSUM accumulation).
"""

import os
import sys

import numpy as np
import ml_dtypes

for _p in ("/opt/trn_rl_repo",):
    if os.path.isdir(_p) and _p not in sys.path:
        sys.path.insert(0, _p)

import concourse.bacc as bacc
import concourse.mybir as mybir
import concourse.tile as tile
from concourse import bass_utils
from concourse.ap import AP

B, T, C, H, D = 2, 2048, 1024, 16, 64
NCORES = 8
GP = 4              # cores per batch group
HPC = H // GP       # heads per core = 4
DS = HPC * D        # per-core head-dim slice = 256
NCT = C // 128      # c-tiles = 8
NQC = T // 512      # q-chunks = 4
NKT = T // 128      # k-tiles = 16

F32 = mybir.dt.float32
BF16 = mybir.dt.bfloat16
AF = mybir.ActivationFunctionType
ALU = mybir.AluOpType
NPBF = ml_dtypes.bfloat16

_PROG = None
LAST_RESULTS = None  # BassKernelResults of the most recent run (for test.py)


def _normalize_recip(nc, pn, ops, hp):
    """Vector-only first half of the pair normalize: stage both heads'
    row-sums and compute one [2, 512] reciprocal (the iterative
    reciprocal costs ~6 cycles per FREE element regardless of
    partitions, so one call covers the pair). Emitted right at pair end
    -- nothing here touches the PE queue."""
    sums = pn.tile([65, 512], F32, tag="sums")
    nc.vector.tensor_copy(sums[64:65, :], ops[hp][64:65, :])
    nc.vector.tensor_copy(sums[0:1, :], ops[hp + 1][64:65, :])
    recip_bf = pn.tile([65, 512], BF16, tag="recipbf")
    with nc.allow_low_precision(reason="normalize denominators"):
        nc.vector.reciprocal(recip_bf[0:65, :], sums[0:65, :])
    return recip_bf


def _normalize_apply(nc, pn, psB, ones_t, attn_sb, ops, recip_bf, hp, qc):
    """Second half: one matmul against the indicator block (row 64 of
    ones_t selects output partitions 0..63 for head hp, row 0 selects
    64..127 for head hp+1) broadcasts the reciprocals, then two
    multiplies write the normalized attention. Deferred into the next
    pair's S-stream so the PE queue never waits on the reciprocal."""
    bc_ps = psB.tile([128, 512], F32, tag="bcs")
    nc.tensor.matmul(
        bc_ps[:, :],
        ones_t[0:65, :],
        recip_bf[0:65, :],
        start=True,
        stop=True,
    )
    bc_sb = pn.tile([128, 512], F32, tag="bc")
    nc.vector.tensor_copy(bc_sb[:, :], bc_ps[:, :])
    for j, h in enumerate((hp, hp + 1)):
        aslc = attn_sb[(h, qc // 2)][:, 512 * (qc % 2) : 512 * (qc % 2 + 1)]
        nc.vector.tensor_tensor(
            aslc, ops[h][0:64, :], bc_sb[64 * j : 64 * (j + 1), :], ALU.mult
        )


def _emit(nc, tc, io):
    (xT, wqkv, wo, bq2, bk2, bo_bc, maskd, onesd, onesv, out_shard) = io

    ag_in = [
        nc.dram_tensor(f"ag_in{i}", [DS, 512], BF16) for i in range(NQC)
    ]
    ag_out = [
        nc.dram_tensor(
            f"ag_out{i}", [NCORES * DS, 512], BF16, addr_space="Shared"
        )
        for i in range(NQC)
    ]

    with (
        tc.tile_pool(name="outer", bufs=1) as po,
    ):
        # ---- persistent tiles; weights ride the scalar HWDGE queue ----
        bq_sb = po.tile([128, 2], F32, tag="bq")
        nc.scalar.dma_start(bq_sb[:, :], bq2[:, :])
        bk_sb = po.tile([128, 2], F32, tag="bk")
        nc.scalar.dma_start(bk_sb[:, :], bk2[:, :])
        ones_t = po.tile([128, 128], BF16, tag="ones")
        nc.scalar.dma_start(ones_t[:, :], onesd[:, :])
        mask_sb = po.tile([128, 4 * 512], BF16, tag="mask")
        attn_sb = {}
        for h in range(HPC):
            for hf in (0, 1):
                attn_sb[(h, hf)] = po.tile(
                    [64, T // 2], BF16, tag=f"attn{h}_{hf}", name=f"attn{h}_{hf}"
                )

        with (
            tc.tile_pool(name="proj", bufs=1) as pp,
            tc.tile_pool(name="work", bufs=16) as pw,
            tc.tile_pool(name="nrm", bufs=2) as pn,
            tc.tile_pool(name="fin", bufs=1) as pf,
            tc.tile_pool(name="ao", bufs=2) as pao,
            tc.tile_pool(name="osb", bufs=3) as posb,
            tc.tile_pool(name="psA", bufs=5, space="PSUM") as psA,
            tc.tile_pool(name="psO", bufs=1, space="PSUM") as psO,
            tc.tile_pool(name="psB", bufs=1, space="PSUM") as psB,
        ):
            # zero-init the pT pool slots: the diagonal exp skips fully
            # masked column prefixes, and the mask multiply's 0 * stale-bits
            # must not see NaN/Inf bit patterns on the first rotation.
            for i in range(16):
                pT_init = pw.tile([128, 512], BF16, tag="pT", name=f"pT_init{i}")
                nc.vector.memset(pT_init[:, :], 0.0)

            # the pair-normalize reciprocal sweeps partitions 0..64 of the
            # sums tiles; rows 1..63 are never written, so memset both pool
            # slots to 1.0 once (1/1=1, zeroed by the indicator matmul; left
            # as bit-garbage they could be NaN/0 and the matmul's 0*Inf
            # would poison the broadcast).
            for i in range(2):
                s_init = pn.tile([65, 512], F32, tag="sums", name=f"sums_init{i}")
                nc.vector.memset(s_init[0:65, :], 1.0)

            # ---- load x^T (sync queue) and packed wqkv (scalar queue),
            # interleaved per c-tile so the Q projection can start while
            # later tiles are still in flight
            xT_sb, w_sb = [], []
            for ci in range(NCT):
                t_ = pp.tile([128, T], BF16, tag=f"xt{ci}", name=f"xt{ci}")
                nc.sync.dma_start(t_[:, :], xT[128 * ci : 128 * (ci + 1), :])
                xT_sb.append(t_)
                t_ = pp.tile([128, 3 * DS], BF16, tag=f"w{ci}", name=f"w{ci}")
                nc.scalar.dma_start(t_[:, :], wqkv[128 * ci : 128 * (ci + 1), :])
                w_sb.append(t_)
            # remaining constants on the scalar queue (needed later)
            vp_sb = pp.tile([128, HPC * NKT * 65], BF16, tag="vp")
            nc.scalar.dma_start(vp_sb[:, :], onesv[:, :])
            nc.scalar.dma_start(mask_sb[:, :], maskd[:, :])
            wo_sb = []
            for ci in range(NCT):
                t_ = pf.tile([128, DS], BF16, tag=f"wo{ci}", name=f"wo{ci}")
                nc.scalar.dma_start(t_[:, :], wo[128 * ci : 128 * (ci + 1), :])
                wo_sb.append(t_)
            bo_sb = pf.tile([128, DS], F32, tag="bo")
            nc.scalar.dma_start(bo_sb[:, :], bo_bc[:, :])

            # ---- Q^T projection, c-tile-outer: the accumulation for all
            # 4 t-chunks of an mt block proceeds as x^T tiles arrive, so
            # the PE starts ~2us into the x^T load. Two [128,1024] PSUM
            # pair-slots hold the 4 chunk accumulators.
            qT_sb = [
                pp.tile([128, T], BF16, tag=f"qT{mt}", name=f"qT{mt}")
                for mt in range(2)
            ]
            kT_sb = [
                pp.tile([128, T], BF16, tag=f"kT{mt}", name=f"kT{mt}")
                for mt in range(2)
            ]
            def q_proj_ci_outer(mt):
                # c-tile-outer: accumulation proceeds as x^T tiles arrive,
                # so the PE starts ~2us into the x^T load (mt=0 only; by
                # mt=1 the DMAs are done and t-chunk-outer is cheaper on
                # PSUM slots).
                slots = [
                    psA.tile([128, 512], F32, tag="sps", name=f"qacc{mt}_{i}")
                    for i in range(NQC)
                ]
                for ci in range(NCT):
                    for tch in range(NQC):
                        nc.tensor.matmul(
                            slots[tch][:, :],
                            w_sb[ci][:, 128 * mt : 128 * (mt + 1)],
                            xT_sb[ci][:, 512 * tch : 512 * (tch + 1)],
                            start=(ci == 0),
                            stop=(ci == NCT - 1),
                        )
                for tch in range(NQC):
                    nc.vector.tensor_scalar_add(
                        qT_sb[mt][:, 512 * tch : 512 * (tch + 1)],
                        slots[tch][:, :],
                        bq_sb[:, mt : mt + 1],
                    )

            def qk_proj_tch(dst, woff, bias, mt):
                for tch in range(NQC):
                    ps = psA.tile([128, 512], F32, tag="sps")
                    for ci in range(NCT):
                        nc.tensor.matmul(
                            ps[:, :],
                            w_sb[ci][:, woff + 128 * mt : woff + 128 * (mt + 1)],
                            xT_sb[ci][:, 512 * tch : 512 * (tch + 1)],
                            start=(ci == 0),
                            stop=(ci == NCT - 1),
                        )
                    nc.vector.tensor_scalar_add(
                        dst[mt][:, 512 * tch : 512 * (tch + 1)],
                        ps[:, :],
                        bias[:, mt : mt + 1],
                    )

            # ---- V projection: out [t, d] + ones column ----------------
            def v_proj_block(tts):
                for tt in tts:
                    ps = psA.tile([128, 512], F32, tag="sps", name=f"vps{tt}")
                    for ci in range(NCT):
                        nc.tensor.matmul(
                            ps[:, 0:DS],
                            xT_sb[ci][:, 128 * tt : 128 * (tt + 1)],
                            w_sb[ci][:, 2 * DS : 3 * DS],
                            start=(ci == 0),
                            stop=(ci == NCT - 1),
                        )
                    for h in range(HPC):
                        nc.vector.tensor_copy(
                            vp_sb[:, 1040 * h + 65 * tt : 1040 * h + 65 * tt + 64],
                            ps[:, 64 * h : 64 * (h + 1)],
                        )

            # ---- causal attention (S^T orientation) --------------------
            # Head pairs share each k-tile step; PV accumulations are
            # emitted as same-bank-adjacent pairs one k-step behind the
            # S/exp front. V-projection t-blocks are interleaved between
            # q-chunks (each chunk only consumes V up to its causal k
            # range), so the attention pipeline fill overlaps V matmuls.
            # normalize + quarter-AllGather for a finished pair; the caller
            # defers this into the NEXT pair's S-stream so the PE queue is
            # never parked behind the reciprocal chain.
            def flush_pair(pend):
                qc, hp, ops, recip_bf = pend
                _normalize_apply(
                    nc, pn, psB, ones_t, attn_sb, ops, recip_bf, hp, qc
                )
                if hp == 2:
                    # both head pairs of this q-chunk are normalized: ship
                    # the quarter AllGather (Shared output = fast path).
                    for h in range(HPC):
                        nc.sync.dma_start(
                            ag_in[qc][64 * h : 64 * (h + 1), :],
                            attn_sb[(h, qc // 2)][
                                :, 512 * (qc % 2) : 512 * (qc % 2 + 1)
                            ],
                        )
                    nc.gpsimd.collective_compute(
                        "AllGather",
                        ALU.bypass,
                        replica_groups=[list(range(NCORES))],
                        ins=[ag_in[qc][:, :]],
                        outs=[ag_out[qc][:, :]],
                    )

            pending = None

            def attn_pair(qc, hp):
                nonlocal pending
                nkt = 4 * qc + 4
                if True:
                    ops = {}
                    for h in (hp, hp + 1):
                        ops[h] = psO.tile(
                            [65, 512], F32, tag=f"ops{h % 2}", name=f"op_q{qc}h{h}"
                        )
                    pTs = {}
                    for kt in range(nkt + 2):
                        if kt == 3 and pending is not None:
                            flush_pair(pending)
                            pending = None
                        if kt < nkt:
                            for h in (hp, hp + 1):
                                mt, pof = h // 2, 64 * (h % 2)
                                qs = qT_sb[mt][pof : pof + 64, 512 * qc : 512 * (qc + 1)]
                                st = psA.tile([128, 512], F32, tag="sps")
                                nc.tensor.matmul(
                                    st[:, :],
                                    kT_sb[mt][pof : pof + 64, 128 * kt : 128 * (kt + 1)],
                                    qs,
                                    start=True,
                                    stop=True,
                                )
                                pT = pw.tile([128, 512], BF16, tag="pT")
                                dm = kt - 4 * qc
                                # diagonal tiles: columns below dm*128 are
                                # fully masked -> skip them in the exp (the
                                # mask multiply zeroes them; the pT pool is
                                # zero-initialized so stale bits are finite)
                                e0 = dm * 128 if dm > 0 else 0
                                nc.scalar.activation(
                                    pT[:, e0:512], st[:, e0:512], AF.Exp, scale=0.125
                                )
                                if dm >= 0:  # mask cols that can hide
                                    mw = (dm + 1) * 128
                                    # alternate engines: the 4 diagonal
                                    # masks arrive back-to-back at chunk
                                    # end and would serialize on one queue
                                    eng = nc.gpsimd if dm % 2 == 0 else nc.vector
                                    eng.tensor_tensor(
                                        pT[:, 0:mw],
                                        pT[:, 0:mw],
                                        mask_sb[:, 512 * dm : 512 * dm + mw],
                                        ALU.mult,
                                    )
                                pTs[(h, kt)] = pT
                        # PV pairs, emitted same-bank adjacent, one pair
                        # of k-steps behind the S/exp front
                        if kt % 2 == 1 and kt - 1 - 2 >= 0:
                            kv = kt - 1 - 2
                            # alternate heads so consecutive accumulations
                            # hit different PSUM banks (same-bank
                            # back-to-back accumulation stalls the PE
                            # pipeline on the read-modify-write)
                            for k2 in (kv, kv + 1):
                                for h in (hp, hp + 1):
                                    nc.tensor.matmul(
                                        ops[h][:, :],
                                        vp_sb[:, 1040 * h + 65 * k2 : 1040 * h + 65 * k2 + 65],
                                        pTs.pop((h, k2))[:, :],
                                        start=(k2 == 0),
                                        stop=(k2 == nkt - 1),
                                    )
                    # drain remaining PV steps (last two k-tiles)
                    for k2 in sorted(set(k for (hh, k) in pTs)):
                        for h in (hp, hp + 1):
                            nc.tensor.matmul(
                                ops[h][:, :],
                                vp_sb[:, 1040 * h + 65 * k2 : 1040 * h + 65 * k2 + 65],
                                pTs.pop((h, k2))[:, :],
                                start=(k2 == 0),
                                stop=(k2 == nkt - 1),
                            )
                    recip_bf = _normalize_recip(nc, pn, ops, hp)
                    pending = (qc, hp, ops, recip_bf)

            # ---- schedule: interleave the remaining projections and V
            # blocks with early attention pairs (pair (0,0) only needs the
            # mt=0 projections and V t-tiles 0..3), so proj matmuls fill
            # the attention pipeline-fill bubbles.
            q_proj_ci_outer(0)
            qk_proj_tch(kT_sb, DS, bk_sb, 0)
            v_proj_block(range(0, 4))
            attn_pair(0, 0)
            qk_proj_tch(qT_sb, 0, bq_sb, 1)
            qk_proj_tch(kT_sb, DS, bk_sb, 1)
            attn_pair(0, 2)
            v_proj_block(range(4, 8))
            attn_pair(1, 0)
            attn_pair(1, 2)
            v_proj_block(range(8, 16))
            attn_pair(2, 0)
            attn_pair(2, 2)
            attn_pair(3, 0)
            attn_pair(3, 2)
            flush_pair(pending)

            # ---- output projection: full T, 256-column slice of Wo over
            # this batch's 1024 gathered features. The batch's rows sit at
            # a per-core offset (0 or 1024 rows); one dynamic-offset 3-dim
            # mega-DMA per quarter pulls all 8 c-tiles (the dynamic-DMA
            # register/scratch preamble is ~2.3us, so amortize it).
            for tg in range(NQC):
                ao = pao.tile([128, NCT * 512], BF16, tag="ao")
                dq = nc.scalar if tg == 3 else nc.sync
                # the mega-DMA below reads ag_out through a dynamic-offset
                # AP, which the dependency tracker may not tie to the
                # collective's write; this static-offset read IS tracked,
                # and the engine queue is in-order, so it fences the
                # dynamic read behind collective completion.
                fence = pao.tile([1, 64], BF16, tag="agfence")
                dq.dma_start(fence[:, :], ag_out[tg][1024:1025, 0:64])
                boff = (dq.partition_id() // GP) * (C * 512)
                base = ag_out[tg][0:128, :]
                in_ap = AP(
                    base.tensor,
                    base.offset + boff,
                    [[512, 128], [128 * 512, NCT], [1, 512]],
                )
                sb = ao[:, :]
                out_ap = AP(
                    sb.tensor, sb.offset, [[NCT * 512, 128], [512, NCT], [1, 512]]
                )
                dq.dma_start(out_ap, in_ap)
                osb = posb.tile([128, 4 * DS], F32, tag="osb")
                for tj in range(4):
                    tt = 4 * tg + tj
                    ps = psO.tile(
                        [128, DS], F32, tag=f"ops{tt % 2}", name=f"out_ps{tt}"
                    )
                    for ci in range(NCT):
                        nc.tensor.matmul(
                            ps[:, :],
                            ao[:, 512 * ci + 128 * tj : 512 * ci + 128 * (tj + 1)],
                            wo_sb[ci][:, :],
                            start=(ci == 0),
                            stop=(ci == NCT - 1),
                        )
                    nc.vector.tensor_tensor(
                        osb[:, DS * tj : DS * (tj + 1)], ps[:, :], bo_sb[:, :],
                        ALU.add,
                    )
                st_base = out_shard[512 * tg : 512 * tg + 128, :]
                st_out = AP(
                    st_base.tensor,
                    st_base.offset,
                    [[DS, 128], [128 * DS, 4], [1, DS]],
                )
                st_sb = osb[:, :]
                st_in = AP(
                    st_sb.tensor, st_sb.offset, [[4 * DS, 128], [DS, 4], [1, DS]]
                )
                nc.sync.dma_start(st_out, st_in)


def _build_program():
    nc = bacc.Bacc(
        "TRN2",
        target_bir_lowering=False,
        debug=False,
        num_devices=NCORES,
    )
    xT = nc.dram_tensor("xT", [C, T], BF16, kind="ExternalInput")
    wqkv = nc.dram_tensor("wqkv", [C, 3 * DS], BF16, kind="ExternalInput")
    wo = nc.dram_tensor("wo", [C, DS], BF16, kind="ExternalInput")
    bq2 = nc.dram_tensor("bq2", [128, 2], F32, kind="ExternalInput")
    bk2 = nc.dram_tensor("bk2", [128, 2], F32, kind="ExternalInput")
    bo_bc = nc.dram_tensor("bo_bc", [128, DS], F32, kind="ExternalInput")
    maskd = nc.dram_tensor("maskd", [128, 4 * 512], BF16, kind="ExternalInput")
    onesd = nc.dram_tensor("onesd", [128, 128], BF16, kind="ExternalInput")
    onesv = nc.dram_tensor(
        "onesv", [128, HPC * NKT * 65], BF16, kind="ExternalInput"
    )
    out_shard = nc.dram_tensor("out_shard", [T, DS], F32, kind="ExternalOutput")
    io = (xT, wqkv, wo, bq2, bk2, bo_bc, maskd, onesd, onesv, out_shard)
    with tile.TileContext(nc) as tc:
        _emit(nc, tc, io)
    nc.compile()
    return nc


def _make_indicator():
    # row 64 selects the low 64 output partitions (head hp) and row 0 the
    # high 64 (head hp+1) for the paired softmax-denominator broadcast
    # matmul; other rows are zero so the stray reciprocal outputs on
    # partitions 1..63 (finite, via the memset-to-1 of the sums tiles)
    # cannot contribute.
    ind = np.zeros((128, 128), np.float32)
    ind[64, 0:64] = 1.0
    ind[0, 64:128] = 1.0
    return ind.astype(NPBF)


def _make_mask():
    # multiplicative causal mask blocks for the 4 diagonal positions:
    # 1 where k is visible (128*m + k_local <= q_local), 0 otherwise
    k = np.arange(128, dtype=np.int64)[:, None]
    q = np.arange(512, dtype=np.int64)[None, :]
    mask = np.zeros((128, 4 * 512), np.float32)
    for m in range(4):
        mask[:, 512 * m : 512 * (m + 1)] = (128 * m + k <= q).astype(np.float32)
    return mask.astype(NPBF)


def _make_in_maps(x, Wq, bq, Wk, bk, Wv, bv, Wo, bo):
    mask = _make_mask()
    in_maps = []
    for c in range(NCORES):
        b, g = c // GP, c % GP
        hs = slice(DS * g, DS * (g + 1))
        wqkv = np.concatenate([Wq[:, hs], Wk[:, hs], Wv[:, hs]], axis=1)
        bo_eff = bo[hs] + bv @ Wo[:, hs]  # V bias folded through Wo
        in_maps.append(
            {
                "xT": np.ascontiguousarray(x[b].T).astype(NPBF),
                "wqkv": np.ascontiguousarray(wqkv).astype(NPBF),
                "wo": np.ascontiguousarray(Wo[:, hs]).astype(NPBF),
                "bq2": np.ascontiguousarray(bq[hs].reshape(2, 128).T),
                "bk2": np.ascontiguousarray(bk[hs].reshape(2, 128).T),
                "bo_bc": np.tile(bo_eff[None, :], (128, 1)).astype(np.float32),
                "maskd": mask,
                "onesd": _make_indicator(),
                "onesv": np.ones((128, HPC * NKT * 65), NPBF),
            }
        )
    return in_maps


def kernel(x, Wq, bq, Wk, bk, Wv, bv, Wo, bo, _trace=False, _trace_cores=None):
    global _PROG, LAST_RESULTS
    x = np.asarray(x, np.float32)
    Wq, bq = np.asarray(Wq, np.float32), np.asarray(bq, np.float32)
    Wk, bk = np.asarray(Wk, np.float32), np.asarray(bk, np.float32)
    Wv, bv = np.asarray(Wv, np.float32), np.asarray(bv, np.float32)
    Wo, bo = np.asarray(Wo, np.float32), np.asarray(bo, np.float32)

    if _PROG is None:
        _PROG = _build_program()
    nc = _PROG

    in_maps = _make_in_maps(x, Wq, bq, Wk, bk, Wv, bv, Wo, bo)

    kw = {}
    if _trace:
        kw["trace"] = True
        if _trace_cores is not None:
            kw["trace_cores"] = _trace_cores
    res = bass_utils.run_bass_kernel_spmd(nc, in_maps, list(range(NCORES)), **kw)
    LAST_RESULTS = res

    out = np.empty((B, T, C), np.float32)
    for c in range(NCORES):
        b, g = c // GP, c % GP
        out[b, :, DS * g : DS * (g + 1)] = res.results[c]["out_shard"]
    return out
